# revision 59
# baseline (speedup 1.0000x reference)
"""Fused multi-head attention on 8 TRN2 NeuronCores.

Problem: x[2,2048,1024] -> q,k,v = x@W.T+b (16 heads x 64), softmax(q k^T/8) v,
then out @ Wp.T + bp.

Sharding: data-parallel over batch (2) x tensor-parallel over heads (4 ranks x
4 heads = 256 dims, Megatron-style).  Core c handles batch c//4, head-rank c%4.
The proj partial sums are reduced on the host (numpy), and the v-bias and
proj-bias are folded into one host-side vector bp_eff = bv @ Wp.T + bp.

Per-core layouts (host pre-transposes/pre-tiles, all DMA rows are >=2KB
contiguous DRAM runs):
  xT  [1024, 2048]  x[b].T (DMA'd in column halves so the h2=0 projection
                    groups start after 2MB instead of 4MB)
  wq3/wk3/wvT [128, 8, 256]  W.T slice pre-tiled so partition p holds all
                             8 contraction tiles contiguously
  wpT [256, 1024]            Wp.T rows for this rank's 256 dims
  bq/bk [256, 1]
  outT [1024, 2048] bf16 partial (x[b] @ ..).T, missing bv/bp contributions

Kernel math per core (all matmul operands bfloat16, fp32 PSUM accumulate):
  qT = wqT.T @ xT + bq   [256, 2048]  (transposed layout, d on partitions)
  kT = wkT.T @ xT + bk   [256, 2048]
  v  = xT.T @ wvT        [2048, 256]  (natural layout, packed per head)
  attention runs as ONE flat software-pipelined stream over all 128
  (n-chunk, head-pair, key-block) blocks; per block:
     sT[m, n] = kT.T @ qT    two heads row-packed in the PE (K=64 tiles)
     p = exp(sT / 8)         ACT, one [128,1024] instr, both heads
     po[d, n]  += v.T @ p    col-packed pair, heads at partitions 0:64/64:128
     pd[d', n] += 1.T @ s2   col-packed all-ones pair = softmax denominators;
                             s2 = DVE pair-sums of adjacent exp tiles, and
                             the matmuls lag those adds by 4 blocks so the
                             in-order PE never stalls on them
  PV/den trail their block's exp by one position globally, so the in-order
  PE always has the next score matmuls queued while ACT runs exp.  The
  q/k/v projections and the output projection are emitted as "filler"
  generators pumped per block inside the stream, load-balanced so the
  outproj-heavy positions shed work into the ACT-paced slack of the
  filler-free positions.
  attnT = po * reciprocal_approx_fast(pd)    one DVE mul per head pair
  outT += wpT.T @ attnT    [1024, n-chunk] per chunk, staged via SBUF
"""

import numpy as np

DIM = 1024
N_TOK = 2048
N_HEADS_LOC = 4       # heads per core
D_LOC = 256           # local q/k/v dims per core
SCALE = 64 ** -0.5
P = 128
CH = 512              # n-chunk (moving free dim)
NCH = N_TOK // CH     # 4
KT = DIM // P         # 8 contraction tiles for qkv/proj
MB = N_TOK // P       # 16 key blocks
N_CORES = 8

_NC_CACHE = {}


def build_nc(dt_mm_name="float32r"):
    import concourse.mybir as mybir
    import concourse.tile as tile
    from concourse import bacc
    from concourse.bass import ts

    f32 = mybir.dt.float32
    dt_mm = getattr(mybir.dt, dt_mm_name)
    fp8 = mybir.dt.float8e4
    DR = mybir.MatmulPerfMode.DoubleRow
    Exp = mybir.ActivationFunctionType.Exp

    nc = bacc.Bacc("TRN2", target_bir_lowering=False, debug=False,
                   num_devices=N_CORES)
    xT = nc.dram_tensor("xT", [DIM, N_TOK], dt_mm, kind="ExternalInput").ap()
    wqd = nc.dram_tensor("wq3", [P, KT, D_LOC], dt_mm, kind="ExternalInput").ap()
    wkd = nc.dram_tensor("wk3", [P, KT, D_LOC], dt_mm, kind="ExternalInput").ap()
    wvT = nc.dram_tensor("wvT", [P, KT * D_LOC], dt_mm, kind="ExternalInput").ap()
    wpT = nc.dram_tensor("wpT", [D_LOC, DIM], dt_mm, kind="ExternalInput").ap()
    bq = nc.dram_tensor("bq", [D_LOC, 1], f32, kind="ExternalInput").ap()
    bk = nc.dram_tensor("bk", [D_LOC, 1], f32, kind="ExternalInput").ap()
    outT = nc.dram_tensor("outT", [DIM, N_TOK], dt_mm, kind="ExternalOutput").ap()

    with tile.TileContext(nc) as tc:
        with (
            tc.tile_pool(name="const", bufs=1) as const,
            tc.tile_pool(name="work", bufs=2) as work,
            tc.tile_pool(name="psum", bufs=3, space="PSUM") as psum,
            tc.tile_pool(name="psum_o", bufs=2, space="PSUM") as psum_o,
        ):
            # All input DMA on ONE queue (gpsimd/Pool: cheapest issues) in
            # exact need-order, so the 16 DMA engines complete transfers in
            # that order instead of starving the weights behind x.  Biases
            # ride the scalar queue (idle until the first exp).  The sync
            # queue stays free for output DMA.
            w8 = {}
            for name, src in (("k", wkd), ("q", wqd)):
                w8[name] = const.tile([P, KT, D_LOC], dt_mm, tag=f"w8{name}",
                                      name=f"w8{name}")
                if name == "k":
                    nc.gpsimd.dma_start(out=w8[name][:], in_=src[:])
            bias_sb = {}
            for name, src_ap in (("q", bq), ("k", bk)):
                bias_sb[name] = []
                for mt in range(D_LOC // P):
                    t = const.tile([P, 1], f32, tag=f"b{name}{mt}",
                                   name=f"b{name}{mt}")
                    nc.scalar.dma_start(out=t[:], in_=src_ap[ts(mt, P), :])
                    bias_sb[name].append(t)
            x_sb = []
            for i in range(KT):
                t = const.tile([P, N_TOK], dt_mm, tag=f"x{i}", name=f"x{i}")
                x_sb.append(t)
            nc.gpsimd.dma_start(out=w8["q"][:], in_=wqd[:])
            for i in range(KT):      # first column halves of every tile
                nc.gpsimd.dma_start(out=x_sb[i][:, 0:1024],
                                    in_=xT[ts(i, P), 0:1024])
            wv_tile = const.tile([P, KT, D_LOC], dt_mm, tag="wv", name="wv")
            nc.gpsimd.dma_start(out=wv_tile[:],
                                in_=wvT[:].rearrange("p (k n) -> p k n", k=KT))
            for i in range(KT):      # second column halves
                nc.gpsimd.dma_start(out=x_sb[i][:, 1024:2048],
                                    in_=xT[ts(i, P), 1024:2048])
            wv_sb = [wv_tile[:, i, :] for i in range(KT)]
            wp_sb = []
            for i in range(D_LOC // P):
                t = const.tile([P, DIM], dt_mm, tag=f"wp{i}", name=f"wp{i}")
                nc.gpsimd.dma_start(out=t[:], in_=wpT[ts(i, P), :])
                wp_sb.append(t)

            ones_sb = const.tile([P, 64], dt_mm, tag="ones")
            nc.vector.memset(ones_sb[:], 1.0)
            warm_sb = const.tile([P, CH], dt_mm, tag="warm")
            nc.vector.memset(warm_sb[:], 0.25)

            qk_sb = {}
            for name in ("q", "k"):
                qk_sb[name] = [
                    const.tile([P, N_TOK], dt_mm, tag=f"{name}T{mt}",
                               name=f"{name}T{mt}")
                    for mt in range(D_LOC // P)
                ]
            vpk_sb = [
                const.tile([P, N_HEADS_LOC, 64], dt_mm, tag=f"vp{nt}",
                           name=f"vp{nt}")
                for nt in range(MB)
            ]
            at_sb = {}

            # ---- emission units; generators double as pipeline fillers ----
            def gen_proj(name, mt, h2, step, halves=(0, 1)):
                """q/k projection group; yields every `step` matmuls."""
                ps = psum.tile([P, 1024], f32, tag="ps",
                               name=f"ps_{name}{mt}{h2}{halves[0]}")
                n = 0
                for half in halves:
                    for kt in range(KT):
                        nc.tensor.matmul(
                            ps[:, ts(half, CH)],
                            lhsT=w8[name][:, kt, ts(mt, P)],
                            rhs=x_sb[kt][:, ts(2 * h2 + half, CH)],
                            start=(kt == 0), stop=(kt == KT - 1),
                        )
                        n += 1
                        if n % step == 0:
                            yield
                    # per-half bias epilogue: downstream QK consumers wait
                    # on 512-col writes instead of the whole 1024-col group
                    nc.vector.tensor_scalar_add(
                        qk_sb[name][mt][:, ts(2 * h2 + half, CH)],
                        ps[:, ts(half, CH)], bias_sb[name][mt][:],
                    )
                yield

            def gen_vproj():
                """One v-projection group (one key block) per yield."""
                for nt in range(MB):
                    ps = psum.tile([P, 1024], f32, tag="ps", name=f"ps_v{nt}")
                    for kt in range(KT):
                        nc.tensor.matmul(
                            ps[:, 0:D_LOC],
                            lhsT=x_sb[kt][:, ts(nt, P)],
                            rhs=wv_sb[kt][:],
                            start=(kt == 0), stop=(kt == KT - 1),
                        )
                    for h in range(N_HEADS_LOC):
                        nc.vector.tensor_copy(vpk_sb[nt][:, h, :],
                                              ps[:, ts(h, 64)])
                    yield

            def gen_outproj(ch, alt_cast=False, dts=(0, 1), dst=None):
                """Output projection for chunk ch; one mo-half per yield.
                Copies + DMA run per 512-col half (bf16) so psum bufs and
                DVE work retire at half granularity.  alt_cast alternates
                the psum->sbuf casts onto the scalar engine (idle in the
                tail) so the last chunk is not DVE-serialized.  dts/dst
                select a partial contraction written to its own output."""
                at_tiles = at_sb[ch]
                for mp in range(DIM // P // 2):
                    pp = psum.tile([P, 1024], f32, tag="ps", name=f"pp{ch}{mp}")
                    for half in range(2):
                        mo = 2 * mp + half
                        for dt_i in dts:
                            nc.tensor.matmul(
                                pp[:, ts(half, CH)],
                                lhsT=wp_sb[dt_i][:, ts(mo, P)],
                                rhs=at_tiles[dt_i][:],
                                start=(dt_i == dts[0]), stop=(dt_i == dts[-1]),
                            )
                        os_sb = work.tile([P, CH], dt_mm, tag="os", bufs=4,
                                          name=f"os{ch}{mp}{half}{dts[0]}")
                        if alt_cast and half == 1:
                            nc.scalar.copy(os_sb[:], pp[:, ts(half, CH)])
                        else:
                            nc.vector.tensor_copy(os_sb[:], pp[:, ts(half, CH)])
                        # the last chunk's DMAs split across two queues so
                        # the final drain overlaps the outproj stream
                        eng = nc.gpsimd if (alt_cast and half == 1) else nc.sync
                        if dst is None:
                            eng.dma_start(out=outT[ts(mo, P), ts(ch, CH)],
                                          in_=os_sb[:])
                        else:
                            eng.dma_start(out=dst[ts(mo, P), :], in_=os_sb[:])
                        yield

            def run(gen):
                for _ in gen:
                    pass

            # ---- flat software-pipelined stream over all key blocks ----
            # QK+exp lead PV/den by one block globally, so the in-order PE
            # always has score work queued while ACT runs exp, including
            # across (chunk, head-pair) boundaries.
            SEQ = [(0, 0), (1, 0), (0, 1), (1, 1),
                   (2, 0), (2, 1), (3, 0), (3, 1)]
            # per-position deferred PE work (must not be consumed in-loop,
            # except vproj/k01 whose consumers trail by the pipeline lag)
            from itertools import chain

            def pos0_filler():
                # one v-projection group per block (PV of block mb consumes
                # vpk[mb] one block later) plus slices of the k h2=1 group
                # (first QK consumer: block 8) and the deferred q(0,0) h1
                # half (first consumer: position 1)
                vp = gen_vproj()
                rest = chain(gen_proj("k", 0, 1, 3),
                             gen_proj("q", 0, 0, 2, halves=(1,)))
                for _ in range(MB):
                    next(vp, None)
                    next(rest, None)
                    yield

            fillers = {
                0: pos0_filler(),
                1: chain(gen_proj("k", 1, 0, 3), gen_proj("q", 1, 0, 3)),
                2: chain(gen_proj("k", 1, 1, 2), gen_proj("q", 0, 1, 2),
                         gen_proj("q", 1, 1, 2)),
                3: None,   # outproj gens assigned below once at_sb exists;
                4: None,   # positions 5 and 7 drain the leftovers in their
                5: None,   # ACT-paced slack
                6: None,
                7: None,
            }

            # minimal pre-stream work: kT[0] for key blocks 0-7 and the
            # first q chunk; everything else streams in as filler.
            # Dependency-free warmup matmuls run first and between the
            # x-gated projection matmuls so the PE clock ramps to full
            # speed and stays there through the input-DMA wait.
            warm_ps = psum.tile([P, 1024], f32, tag="ps", name="warm")

            def gen_warm(n):
                for _ in range(n):
                    nc.tensor.matmul(warm_ps[0:64, 0:CH], lhsT=ones_sb[:],
                                     rhs=warm_sb[:], start=True, stop=True)
                    yield

            wg = gen_warm(16)
            for _ in range(8):
                next(wg, None)
            # one warm matmul between each x-piece-gated k matmul absorbs
            # the DMA-arrival gaps and keeps the PE clock ramped
            for _ in gen_proj("k", 0, 0, 1, halves=(0,)):
                next(wg, None)
            run(gen_proj("q", 0, 0, 16, halves=(0,)))
            run(gen_proj("k", 0, 0, 16, halves=(1,)))

            blocks = [(i, c, h, mb) for i, (c, h) in enumerate(SEQ)
                      for mb in range(MB)]
            pts = {}
            pend = {}
            po_pd = {}
            for g in range(len(blocks) + 1):
                if g < len(blocks):
                    i, c, h, mb = blocks[g]
                    if mb == 0:
                        if i == 3:
                            op0 = gen_outproj(0)
                            fillers[3] = op0
                        elif i == 4:
                            op1 = gen_outproj(1)
                            fillers[4] = op1
                        elif i == 5:
                            fillers[5] = chain(op0, op1)
                        elif i == 6:
                            op2 = gen_outproj(2)
                            fillers[6] = op2
                        elif i == 7:
                            fillers[7] = op2
                        po_pd[(c, h)] = (
                            psum_o.tile([P, CH], f32, tag="po", name=f"po{c}{h}"),
                            psum_o.tile([P, CH], f32, tag="po", name=f"pd{c}{h}"),
                        )
                    f = fillers.get(i)
                    # scores + exp lead; the filler slice runs on the PE
                    # while ACT computes the exp
                    ps = psum.tile([P, 1024], f32, tag="ps", name=f"s{c}{h}{mb}")
                    nc.tensor.matmul(
                        ps[:, 0:CH],
                        lhsT=qk_sb["k"][h][0:64, ts(mb, P)],
                        rhs=qk_sb["q"][h][0:64, ts(c, CH)],
                    )
                    nc.tensor.matmul(
                        ps[:, CH:1024],
                        lhsT=qk_sb["k"][h][64:P, ts(mb, P)],
                        rhs=qk_sb["q"][h][64:P, ts(c, CH)],
                    )
                    pt = work.tile([P, 1024], dt_mm, tag="pt", bufs=8,
                                   name=f"pt{c}{h}{mb}")
                    nc.scalar.activation(pt[:], ps[:], Exp, scale=SCALE)
                    pts[(c, h, mb)] = pt
                if g >= 1:
                    i2, c2, h2, mb2 = blocks[g - 1]
                    po, pd = po_pd[(c2, h2)]
                    pt = pts[(c2, h2, mb2)]
                    st = (mb2 == 0)
                    sp = (mb2 == MB - 1)
                    nc.tensor.matmul(
                        po[0:64, :], lhsT=vpk_sb[mb2][:, 2 * h2, :],
                        rhs=pt[:, 0:CH], start=st, stop=sp,
                    )
                    nc.tensor.matmul(
                        po[64:P, :], lhsT=vpk_sb[mb2][:, 2 * h2 + 1, :],
                        rhs=pt[:, CH:1024], start=st, stop=sp,
                    )
                    # denominators: adjacent exp tiles pair-summed on the
                    # DVE engine; denominator matmuls lag the adds by 4
                    # blocks so the in-order PE never stalls on them; last
                    # two blocks use raw tiles
                    if mb2 % 2 == 1 and mb2 <= MB - 3:
                        pt0 = pts.pop((c2, h2, mb2 - 1))
                        pts2 = work.tile([P, 1024], dt_mm, tag="pts2", bufs=6,
                                         name=f"pts2_{c2}{h2}{mb2}")
                        nc.vector.tensor_add(pts2[:], pt0[:], pt[:])
                        pend.setdefault((c2, h2), []).append(pts2)
                        pts.pop((c2, h2, mb2))
                    if mb2 % 2 == 1 and 5 <= mb2:
                        s2 = pend[(c2, h2)].pop(0)
                        nc.tensor.matmul(
                            pd[0:64, :], lhsT=ones_sb[:],
                            rhs=s2[:, 0:CH], start=(mb2 == 5), stop=False,
                        )
                        nc.tensor.matmul(
                            pd[64:P, :], lhsT=ones_sb[:],
                            rhs=s2[:, CH:1024], start=(mb2 == 5), stop=False,
                        )
                    if sp:
                        for s2 in pend.pop((c2, h2)):
                            nc.tensor.matmul(
                                pd[0:64, :], lhsT=ones_sb[:],
                                rhs=s2[:, 0:CH], start=False, stop=False,
                            )
                            nc.tensor.matmul(
                                pd[64:P, :], lhsT=ones_sb[:],
                                rhs=s2[:, CH:1024], start=False, stop=False,
                            )
                        for mbx in (MB - 2, MB - 1):
                            ptx = pts.pop((c2, h2, mbx))
                            nc.tensor.matmul(
                                pd[0:64, :], lhsT=ones_sb[:],
                                rhs=ptx[:, 0:CH], start=False,
                                stop=(mbx == MB - 1),
                            )
                            nc.tensor.matmul(
                                pd[64:P, :], lhsT=ones_sb[:],
                                rhs=ptx[:, CH:1024], start=False,
                                stop=(mbx == MB - 1),
                            )
                        # normalize: cast po to sbuf first (frees its psum
                        # bank for the next pair's PV immediately), then
                        # reciprocal + multiply
                        del po_pd[(c2, h2)]
                        poS = work.tile([P, CH], dt_mm, tag="poS", bufs=2,
                                        name=f"poS{c2}{h2}")
                        nc.vector.tensor_copy(poS[:], po[:])
                        rec = work.tile([P, CH], f32, tag="bc", bufs=4,
                                        name=f"rec{c2}{h2}")
                        nc.vector.reciprocal_approx_fast(rec[:], pd[:])
                        at = work.tile([P, CH], dt_mm, tag="at", bufs=4,
                                       name=f"at{c2}{h2}")
                        nc.vector.tensor_mul(at[:], poS[:], rec[:])
                        at_sb.setdefault(c2, []).append(at)
                # filler slice last: it overlaps this block's exp on the
                # PE without delaying the PV/den/normalize emission above.
                # outproj positions (3,4,6) consume only every 4th block;
                # the leftovers drain in the ACT-paced slack of 5 and 7.
                if g < len(blocks):
                    if f is not None and (i < 3 or mb >= 1):
                        if i not in (3, 4, 6) or mb % 4 == 1:
                            next(f, None)
                    if mb == MB - 1 and f is not None and i not in (3, 4, 6):
                        run(f)   # drain deferred work before leaving position
            run(gen_outproj(3, alt_cast=True))

    nc.compile()
    return nc


def _get_nc():
    if "nc" not in _NC_CACHE:
        _NC_CACHE["nc"] = build_nc(DT_MM_NAME)
    return _NC_CACHE["nc"]


def make_in_maps(x, Wq, bq, Wk, bk, Wv, bv, Wp, bp, dt_mm_name="float32r"):
    """Shard full inputs into 8 per-core input maps."""
    import ml_dtypes
    f = np.float32
    if dt_mm_name == "bfloat16":
        mmt = ml_dtypes.bfloat16
    else:
        mmt = np.float32
    x = np.asarray(x, f)
    xT = [np.ascontiguousarray(x[b].T).astype(mmt) for b in range(x.shape[0])]
    WqT = np.asarray(Wq, f).T
    WkT = np.asarray(Wk, f).T
    WvT = np.asarray(Wv, f).T
    WpT = np.asarray(Wp, f).T
    def pretile(w, t):
        # [1024, 256] -> [128, 8, 256]: partition p holds all 8 k-tiles
        # contiguously so DMA descriptors are long DRAM runs
        return np.ascontiguousarray(
            w.reshape(KT, P, D_LOC).transpose(1, 0, 2)).astype(t)

    in_maps = []
    for c in range(N_CORES):
        b, r = divmod(c, 4)
        sl = slice(D_LOC * r, D_LOC * (r + 1))
        in_maps.append({
            "xT": xT[b],
            "wq3": pretile(WqT[:, sl], mmt),
            "wk3": pretile(WkT[:, sl], mmt),
            "wvT": pretile(WvT[:, sl], mmt).reshape(P, KT * D_LOC).copy(),
            "wpT": np.ascontiguousarray(WpT[sl, :]).astype(mmt),
            "bq": np.asarray(bq, f)[sl].reshape(D_LOC, 1).copy(),
            "bk": np.asarray(bk, f)[sl].reshape(D_LOC, 1).copy(),
        })
    return in_maps


def assemble_output(results, Wv, bv, Wp, bp):
    """Sum TP partials, transpose back, add folded biases."""
    f = np.float32
    bp_eff = np.asarray(bv, f) @ np.asarray(Wp, f).T + np.asarray(bp, f)
    out = np.empty((2, N_TOK, DIM), f)
    for b in range(2):
        acc = results[4 * b]["outT"].astype(f)
        for r in range(1, 4):
            acc = acc + results[4 * b + r]["outT"].astype(f)
        out[b] = acc.T + bp_eff
    return out


DT_MM_NAME = "bfloat16"


def kernel(x, Wq, bq, Wk, bk, Wv, bv, Wp, bp):
    from concourse.bass_utils import run_bass_kernel_spmd
    nc = _get_nc()
    in_maps = make_in_maps(x, Wq, bq, Wk, bk, Wv, bv, Wp, bp, DT_MM_NAME)
    res = run_bass_kernel_spmd(nc, in_maps, list(range(N_CORES)))
    return assemble_output(res.results, Wv, bv, Wp, bp)



# revision 61
# speedup vs baseline: 1.2088x; 1.2088x over previous
"""Fused multi-head attention on 8 TRN2 NeuronCores.

Problem: x[2,2048,1024] -> q,k,v = x@W.T+b (16 heads x 64), softmax(q k^T/8) v,
then out @ Wp.T + bp.

Sharding: data-parallel over batch (2) x tensor-parallel over heads (4 ranks x
4 heads = 256 dims, Megatron-style).  Core c handles batch c//4, head-rank c%4.
The proj partial sums are reduced on the host (numpy), and the v-bias and
proj-bias are folded into one host-side vector bp_eff = bv @ Wp.T + bp.

Per-core layouts (host pre-transposes/pre-tiles, all DMA rows are >=2KB
contiguous DRAM runs):
  xT  [1024, 2048]  x[b].T (DMA'd in column halves so the h2=0 projection
                    groups start after 2MB instead of 4MB)
  wq3/wk3/wvT [128, 8, 256]  W.T slice pre-tiled so partition p holds all
                             8 contraction tiles contiguously
  wpT [256, 1024]            Wp.T rows for this rank's 256 dims
  bq/bk [256, 1]
  outT [1024, 2048] bf16 partial (x[b] @ ..).T, missing bv/bp contributions

Kernel math per core (all matmul operands bfloat16, fp32 PSUM accumulate):
  qT = wqT.T @ xT + bq   [256, 2048]  (transposed layout, d on partitions)
  kT = wkT.T @ xT + bk   [256, 2048]
  v  = xT.T @ wvT        [2048, 256]  (natural layout, packed per head)
  attention runs as ONE flat software-pipelined stream over all 128
  (n-chunk, head-pair, key-block) blocks; per block:
     sT[m, n] = kT.T @ qT    two heads row-packed in the PE (K=64 tiles)
     p = exp(sT / 8)         ACT, one [128,1024] instr, both heads
     po[d, n]  += v.T @ p    col-packed pair, heads at partitions 0:64/64:128
     pd[d', n] += 1.T @ s2   col-packed all-ones pair = softmax denominators;
                             s2 = DVE pair-sums of adjacent exp tiles, and
                             the matmuls lag those adds by 4 blocks so the
                             in-order PE never stalls on them
  PV/den trail their block's exp by one position globally, so the in-order
  PE always has the next score matmuls queued while ACT runs exp.  The
  q/k/v projections and the output projection are emitted as "filler"
  generators pumped per block inside the stream, load-balanced so the
  outproj-heavy positions shed work into the ACT-paced slack of the
  filler-free positions.
  attnT = po * reciprocal_approx_fast(pd)    one DVE mul per head pair
  outT += wpT.T @ attnT    [1024, n-chunk] per chunk, staged via SBUF
"""

import numpy as np

DIM = 1024
N_TOK = 2048
N_HEADS_LOC = 4       # heads per core
D_LOC = 256           # local q/k/v dims per core
SCALE = 64 ** -0.5
P = 128
CH = 512              # n-chunk (moving free dim)
NCH = N_TOK // CH     # 4
KT = DIM // P         # 8 contraction tiles for qkv/proj
MB = N_TOK // P       # 16 key blocks
N_CORES = 8

_NC_CACHE = {}


def build_nc(dt_mm_name="float32r"):
    import concourse.mybir as mybir
    import concourse.tile as tile
    from concourse import bacc
    from concourse.bass import ts

    f32 = mybir.dt.float32
    dt_mm = getattr(mybir.dt, dt_mm_name)
    fp8 = mybir.dt.float8e4
    DR = mybir.MatmulPerfMode.DoubleRow
    Exp = mybir.ActivationFunctionType.Exp

    nc = bacc.Bacc("TRN2", target_bir_lowering=False, debug=False,
                   num_devices=N_CORES)
    xT = nc.dram_tensor("xT", [DIM, N_TOK], dt_mm, kind="ExternalInput").ap()
    wqd = nc.dram_tensor("wq3", [P, KT, D_LOC], dt_mm, kind="ExternalInput").ap()
    wkd = nc.dram_tensor("wk3", [P, KT, D_LOC], dt_mm, kind="ExternalInput").ap()
    wvT = nc.dram_tensor("wvT", [P, KT * D_LOC], dt_mm, kind="ExternalInput").ap()
    wpT = nc.dram_tensor("wpT", [D_LOC, DIM], dt_mm, kind="ExternalInput").ap()
    bq = nc.dram_tensor("bq", [D_LOC, 1], f32, kind="ExternalInput").ap()
    bk = nc.dram_tensor("bk", [D_LOC, 1], f32, kind="ExternalInput").ap()
    outT = nc.dram_tensor("outT", [DIM, N_TOK], dt_mm, kind="ExternalOutput").ap()

    with tile.TileContext(nc) as tc:
        with (
            tc.tile_pool(name="const", bufs=1) as const,
            tc.tile_pool(name="work", bufs=2) as work,
            tc.tile_pool(name="psum", bufs=3, space="PSUM") as psum,
            tc.tile_pool(name="psum_o", bufs=2, space="PSUM") as psum_o,
        ):
            # All input DMA on ONE queue (gpsimd/Pool: cheapest issues) in
            # exact need-order, so the 16 DMA engines complete transfers in
            # that order instead of starving the weights behind x.  Biases
            # ride the scalar queue (idle until the first exp).  The sync
            # queue stays free for output DMA.
            w8 = {}
            for name, src in (("k", wkd), ("q", wqd)):
                w8[name] = const.tile([P, KT, D_LOC], dt_mm, tag=f"w8{name}",
                                      name=f"w8{name}")
                if name == "k":
                    nc.gpsimd.dma_start(out=w8[name][:], in_=src[:])
            bias_sb = {}
            for name, src_ap in (("q", bq), ("k", bk)):
                bias_sb[name] = []
                for mt in range(D_LOC // P):
                    t = const.tile([P, 1], f32, tag=f"b{name}{mt}",
                                   name=f"b{name}{mt}")
                    nc.scalar.dma_start(out=t[:], in_=src_ap[ts(mt, P), :])
                    bias_sb[name].append(t)
            x_sb = []
            for i in range(KT):
                t = const.tile([P, N_TOK], dt_mm, tag=f"x{i}", name=f"x{i}")
                x_sb.append(t)
            nc.gpsimd.dma_start(out=w8["q"][:], in_=wqd[:])
            for i in range(KT):      # first column halves of every tile
                nc.gpsimd.dma_start(out=x_sb[i][:, 0:1024],
                                    in_=xT[ts(i, P), 0:1024])
            wv_tile = const.tile([P, KT, D_LOC], dt_mm, tag="wv", name="wv")
            nc.gpsimd.dma_start(out=wv_tile[:],
                                in_=wvT[:].rearrange("p (k n) -> p k n", k=KT))
            for i in range(KT):      # second column halves
                nc.gpsimd.dma_start(out=x_sb[i][:, 1024:2048],
                                    in_=xT[ts(i, P), 1024:2048])
            wv_sb = [wv_tile[:, i, :] for i in range(KT)]
            wp_sb = []
            for i in range(D_LOC // P):
                t = const.tile([P, DIM], dt_mm, tag=f"wp{i}", name=f"wp{i}")
                nc.gpsimd.dma_start(out=t[:], in_=wpT[ts(i, P), :])
                wp_sb.append(t)

            ones_sb = const.tile([P, 64], dt_mm, tag="ones")
            nc.vector.memset(ones_sb[:], 1.0)
            warm_sb = const.tile([P, CH], dt_mm, tag="warm")
            nc.vector.memset(warm_sb[:], 0.25)

            qk_sb = {}
            for name in ("q", "k"):
                qk_sb[name] = [
                    const.tile([P, N_TOK], dt_mm, tag=f"{name}T{mt}",
                               name=f"{name}T{mt}")
                    for mt in range(D_LOC // P)
                ]
            vpk_sb = [
                const.tile([P, N_HEADS_LOC, 64], dt_mm, tag=f"vp{nt}",
                           name=f"vp{nt}")
                for nt in range(MB)
            ]
            at_sb = {}

            # ---- emission units; generators double as pipeline fillers ----
            def gen_proj(name, mt, h2, step, halves=(0, 1)):
                """q/k projection group; yields every `step` matmuls."""
                ps = psum.tile([P, 1024], f32, tag="ps",
                               name=f"ps_{name}{mt}{h2}{halves[0]}")
                n = 0
                for half in halves:
                    for kt in range(KT):
                        nc.tensor.matmul(
                            ps[:, ts(half, CH)],
                            lhsT=w8[name][:, kt, ts(mt, P)],
                            rhs=x_sb[kt][:, ts(2 * h2 + half, CH)],
                            start=(kt == 0), stop=(kt == KT - 1),
                        )
                        n += 1
                        if n % step == 0:
                            yield
                    # per-half bias epilogue: downstream QK consumers wait
                    # on 512-col writes instead of the whole 1024-col group
                    nc.vector.tensor_scalar_add(
                        qk_sb[name][mt][:, ts(2 * h2 + half, CH)],
                        ps[:, ts(half, CH)], bias_sb[name][mt][:],
                    )
                yield

            def gen_vproj():
                """One v-projection group (one key block) per yield."""
                for nt in range(MB):
                    ps = psum.tile([P, 1024], f32, tag="ps", name=f"ps_v{nt}")
                    for kt in range(KT):
                        nc.tensor.matmul(
                            ps[:, 0:D_LOC],
                            lhsT=x_sb[kt][:, ts(nt, P)],
                            rhs=wv_sb[kt][:],
                            start=(kt == 0), stop=(kt == KT - 1),
                        )
                    for h in range(N_HEADS_LOC):
                        nc.vector.tensor_copy(vpk_sb[nt][:, h, :],
                                              ps[:, ts(h, 64)])
                    yield

            def gen_outproj(ch, alt_cast=False, dts=(0, 1), dst=None):
                """Output projection for chunk ch; one mo-half per yield.
                Copies + DMA run per 512-col half (bf16) so psum bufs and
                DVE work retire at half granularity.  alt_cast alternates
                the psum->sbuf casts onto the scalar engine (idle in the
                tail) so the last chunk is not DVE-serialized.  dts/dst
                select a partial contraction written to its own output."""
                at_tiles = at_sb[ch]
                for mp in range(DIM // P // 2):
                    pp = psum.tile([P, 1024], f32, tag="ps", name=f"pp{ch}{mp}")
                    for half in range(2):
                        mo = 2 * mp + half
                        for dt_i in dts:
                            nc.tensor.matmul(
                                pp[:, ts(half, CH)],
                                lhsT=wp_sb[dt_i][:, ts(mo, P)],
                                rhs=at_tiles[dt_i][:],
                                start=(dt_i == dts[0]), stop=(dt_i == dts[-1]),
                            )
                        os_sb = work.tile([P, CH], dt_mm, tag="os", bufs=4,
                                          name=f"os{ch}{mp}{half}{dts[0]}")
                        if alt_cast and half == 1:
                            nc.scalar.copy(os_sb[:], pp[:, ts(half, CH)])
                        else:
                            nc.vector.tensor_copy(os_sb[:], pp[:, ts(half, CH)])
                        # the last chunk's DMAs split across two queues so
                        # the final drain overlaps the outproj stream
                        eng = nc.gpsimd if (alt_cast and half == 1) else nc.sync
                        if dst is None:
                            eng.dma_start(out=outT[ts(mo, P), ts(ch, CH)],
                                          in_=os_sb[:])
                        else:
                            eng.dma_start(out=dst[ts(mo, P), :], in_=os_sb[:])
                        yield

            def run(gen):
                for _ in gen:
                    pass

            # ---- flat software-pipelined stream over all key blocks ----
            # QK+exp lead PV/den by one block globally, so the in-order PE
            # always has score work queued while ACT runs exp, including
            # across (chunk, head-pair) boundaries.
            SEQ = [(0, 0), (1, 0), (0, 1), (1, 1),
                   (2, 0), (2, 1), (3, 0), (3, 1)]
            # per-position deferred PE work (must not be consumed in-loop,
            # except vproj/k01 whose consumers trail by the pipeline lag)
            from itertools import chain

            def pos0_filler():
                # one v-projection group per block (PV of block mb consumes
                # vpk[mb] one block later) plus slices of the k h2=1 group
                # (first QK consumer: block 8) and the deferred q(0,0) h1
                # half (first consumer: position 1)
                vp = gen_vproj()
                rest = chain(gen_proj("k", 0, 1, 3),
                             gen_proj("q", 0, 0, 2, halves=(1,)))
                for _ in range(MB):
                    next(vp, None)
                    next(rest, None)
                    yield

            fillers = {
                0: pos0_filler(),
                1: chain(gen_proj("k", 1, 0, 3), gen_proj("q", 1, 0, 3)),
                2: chain(gen_proj("k", 1, 1, 2), gen_proj("q", 0, 1, 2),
                         gen_proj("q", 1, 1, 2)),
                3: None,   # outproj gens assigned below once at_sb exists;
                4: None,   # positions 5 and 7 drain the leftovers in their
                5: None,   # ACT-paced slack
                6: None,
                7: None,
            }

            # minimal pre-stream work: kT[0] for key blocks 0-7 and the
            # first q chunk; everything else streams in as filler.
            # Dependency-free warmup matmuls run first and between the
            # x-gated projection matmuls so the PE clock ramps to full
            # speed and stays there through the input-DMA wait.
            warm_ps = psum.tile([P, 1024], f32, tag="ps", name="warm")

            def gen_warm(n):
                for _ in range(n):
                    nc.tensor.matmul(warm_ps[0:64, 0:CH], lhsT=ones_sb[:],
                                     rhs=warm_sb[:], start=True, stop=True)
                    yield

            wg = gen_warm(16)
            for _ in range(8):
                next(wg, None)
            # one warm matmul between each x-piece-gated k matmul absorbs
            # the DMA-arrival gaps and keeps the PE clock ramped
            for _ in gen_proj("k", 0, 0, 1, halves=(0,)):
                next(wg, None)
            run(gen_proj("q", 0, 0, 16, halves=(0,)))
            run(gen_proj("k", 0, 0, 16, halves=(1,)))

            blocks = [(i, c, h, mb) for i, (c, h) in enumerate(SEQ)
                      for mb in range(MB)]
            pts = {}
            pend = {}
            po_pd = {}
            for g in range(len(blocks) + 1):
                if g < len(blocks):
                    i, c, h, mb = blocks[g]
                    if mb == 0:
                        if i == 3:
                            op0 = gen_outproj(0)
                            fillers[3] = op0
                        elif i == 4:
                            op1 = gen_outproj(1)
                            fillers[4] = op1
                        elif i == 5:
                            fillers[5] = chain(op0, op1)
                        elif i == 6:
                            op2 = gen_outproj(2)
                            fillers[6] = op2
                        elif i == 7:
                            fillers[7] = op2
                        po_pd[(c, h)] = (
                            psum_o.tile([P, CH], f32, tag="po", name=f"po{c}{h}"),
                            psum_o.tile([P, CH], f32, tag="po", name=f"pd{c}{h}"),
                        )
                    f = fillers.get(i)
                    # scores + exp lead; the filler slice runs on the PE
                    # while ACT computes the exp
                    ps = psum.tile([P, 1024], f32, tag="ps", name=f"s{c}{h}{mb}")
                    nc.tensor.matmul(
                        ps[:, 0:CH],
                        lhsT=qk_sb["k"][h][0:64, ts(mb, P)],
                        rhs=qk_sb["q"][h][0:64, ts(c, CH)],
                    )
                    nc.tensor.matmul(
                        ps[:, CH:1024],
                        lhsT=qk_sb["k"][h][64:P, ts(mb, P)],
                        rhs=qk_sb["q"][h][64:P, ts(c, CH)],
                    )
                    pt = work.tile([P, 1024], dt_mm, tag="pt", bufs=8,
                                   name=f"pt{c}{h}{mb}")
                    nc.scalar.activation(pt[:], ps[:], Exp, scale=SCALE)
                    pts[(c, h, mb)] = pt
                if g >= 1:
                    i2, c2, h2, mb2 = blocks[g - 1]
                    po, pd = po_pd[(c2, h2)]
                    pt = pts[(c2, h2, mb2)]
                    st = (mb2 == 0)
                    sp = (mb2 == MB - 1)
                    nc.tensor.matmul(
                        po[0:64, :], lhsT=vpk_sb[mb2][:, 2 * h2, :],
                        rhs=pt[:, 0:CH], start=st, stop=sp,
                    )
                    nc.tensor.matmul(
                        po[64:P, :], lhsT=vpk_sb[mb2][:, 2 * h2 + 1, :],
                        rhs=pt[:, CH:1024], start=st, stop=sp,
                    )
                    # denominators: adjacent exp tiles pair-summed on the
                    # DVE engine; denominator matmuls lag the adds by 4
                    # blocks so the in-order PE never stalls on them; last
                    # two blocks use raw tiles
                    if mb2 % 2 == 1 and mb2 <= MB - 3:
                        pt0 = pts.pop((c2, h2, mb2 - 1))
                        pts2 = work.tile([P, 1024], dt_mm, tag="pts2", bufs=6,
                                         name=f"pts2_{c2}{h2}{mb2}")
                        nc.vector.tensor_add(pts2[:], pt0[:], pt[:])
                        pend.setdefault((c2, h2), []).append(pts2)
                        pts.pop((c2, h2, mb2))
                    if mb2 in (3, 7, 11):
                        lvl1 = pend[(c2, h2)]
                        s2b = lvl1.pop()
                        s2a = lvl1.pop()
                        pts4 = work.tile([P, 1024], dt_mm, tag="pts4", bufs=4,
                                         name=f"pts4_{c2}{h2}{mb2}")
                        nc.vector.tensor_add(pts4[:], s2a[:], s2b[:])
                        pend.setdefault((c2, h2, "g"), []).append(pts4)
                    if mb2 in (9, 11, 13):
                        s4 = pend[(c2, h2, "g")].pop(0)
                        nc.tensor.matmul(
                            pd[0:64, :], lhsT=ones_sb[:],
                            rhs=s4[:, 0:CH], start=(mb2 == 9), stop=False,
                        )
                        nc.tensor.matmul(
                            pd[64:P, :], lhsT=ones_sb[:],
                            rhs=s4[:, CH:1024], start=(mb2 == 9), stop=False,
                        )
                    if sp:
                        del pend[(c2, h2, "g")]
                        for s2 in pend.pop((c2, h2)):
                            nc.tensor.matmul(
                                pd[0:64, :], lhsT=ones_sb[:],
                                rhs=s2[:, 0:CH], start=False, stop=False,
                            )
                            nc.tensor.matmul(
                                pd[64:P, :], lhsT=ones_sb[:],
                                rhs=s2[:, CH:1024], start=False, stop=False,
                            )
                        for mbx in (MB - 2, MB - 1):
                            ptx = pts.pop((c2, h2, mbx))
                            nc.tensor.matmul(
                                pd[0:64, :], lhsT=ones_sb[:],
                                rhs=ptx[:, 0:CH], start=False,
                                stop=(mbx == MB - 1),
                            )
                            nc.tensor.matmul(
                                pd[64:P, :], lhsT=ones_sb[:],
                                rhs=ptx[:, CH:1024], start=False,
                                stop=(mbx == MB - 1),
                            )
                        # normalize: cast po to sbuf first (frees its psum
                        # bank for the next pair's PV immediately), then
                        # reciprocal + multiply
                        del po_pd[(c2, h2)]
                        poS = work.tile([P, CH], dt_mm, tag="poS", bufs=2,
                                        name=f"poS{c2}{h2}")
                        nc.vector.tensor_copy(poS[:], po[:])
                        rec = work.tile([P, CH], f32, tag="bc", bufs=4,
                                        name=f"rec{c2}{h2}")
                        nc.vector.reciprocal_approx_fast(rec[:], pd[:])
                        at = work.tile([P, CH], dt_mm, tag="at", bufs=4,
                                       name=f"at{c2}{h2}")
                        nc.vector.tensor_mul(at[:], poS[:], rec[:])
                        at_sb.setdefault(c2, []).append(at)
                # filler slice last: it overlaps this block's exp on the
                # PE without delaying the PV/den/normalize emission above.
                # outproj positions (3,4,6) consume only every 4th block;
                # the leftovers drain in the ACT-paced slack of 5 and 7.
                if g < len(blocks):
                    if f is not None and (i < 3 or mb >= 1):
                        if i not in (3, 4, 6) or mb % 4 == 1:
                            next(f, None)
                    if mb == MB - 1 and f is not None and i not in (3, 4, 6):
                        run(f)   # drain deferred work before leaving position
            run(gen_outproj(3, alt_cast=True))

    nc.compile()
    return nc


def _get_nc():
    if "nc" not in _NC_CACHE:
        _NC_CACHE["nc"] = build_nc(DT_MM_NAME)
    return _NC_CACHE["nc"]


def make_in_maps(x, Wq, bq, Wk, bk, Wv, bv, Wp, bp, dt_mm_name="float32r"):
    """Shard full inputs into 8 per-core input maps."""
    import ml_dtypes
    f = np.float32
    if dt_mm_name == "bfloat16":
        mmt = ml_dtypes.bfloat16
    else:
        mmt = np.float32
    x = np.asarray(x, f)
    xT = [np.ascontiguousarray(x[b].T).astype(mmt) for b in range(x.shape[0])]
    WqT = np.asarray(Wq, f).T
    WkT = np.asarray(Wk, f).T
    WvT = np.asarray(Wv, f).T
    WpT = np.asarray(Wp, f).T
    def pretile(w, t):
        # [1024, 256] -> [128, 8, 256]: partition p holds all 8 k-tiles
        # contiguously so DMA descriptors are long DRAM runs
        return np.ascontiguousarray(
            w.reshape(KT, P, D_LOC).transpose(1, 0, 2)).astype(t)

    in_maps = []
    for c in range(N_CORES):
        b, r = divmod(c, 4)
        sl = slice(D_LOC * r, D_LOC * (r + 1))
        in_maps.append({
            "xT": xT[b],
            "wq3": pretile(WqT[:, sl], mmt),
            "wk3": pretile(WkT[:, sl], mmt),
            "wvT": pretile(WvT[:, sl], mmt).reshape(P, KT * D_LOC).copy(),
            "wpT": np.ascontiguousarray(WpT[sl, :]).astype(mmt),
            "bq": np.asarray(bq, f)[sl].reshape(D_LOC, 1).copy(),
            "bk": np.asarray(bk, f)[sl].reshape(D_LOC, 1).copy(),
        })
    return in_maps


def assemble_output(results, Wv, bv, Wp, bp):
    """Sum TP partials, transpose back, add folded biases."""
    f = np.float32
    bp_eff = np.asarray(bv, f) @ np.asarray(Wp, f).T + np.asarray(bp, f)
    out = np.empty((2, N_TOK, DIM), f)
    for b in range(2):
        acc = results[4 * b]["outT"].astype(f)
        for r in range(1, 4):
            acc = acc + results[4 * b + r]["outT"].astype(f)
        out[b] = acc.T + bp_eff
    return out


DT_MM_NAME = "bfloat16"


def kernel(x, Wq, bq, Wk, bk, Wv, bv, Wp, bp):
    from concourse.bass_utils import run_bass_kernel_spmd
    nc = _get_nc()
    in_maps = make_in_maps(x, Wq, bq, Wk, bk, Wv, bv, Wp, bp, DT_MM_NAME)
    res = run_bass_kernel_spmd(nc, in_maps, list(range(N_CORES)))
    return assemble_output(res.results, Wv, bv, Wp, bp)

